# revision 11
# baseline (speedup 1.0000x reference)
"""Block-diagonal linear (segment_reduce) Trainium2 kernel.

y[b, o] = sum_k x[b, o*16 + k] * weight[o, k]
x: (8192, 32768) f32, weight: (2048, 16) f32 -> y: (8192, 2048) f32

Sharding: data-parallel over batch across 8 NeuronCores (1024 rows each);
weight replicated (broadcast across partitions on-chip by the otherwise-idle
TensorE instead of re-reading it 128x from HBM). Per core the kernel streams
x in (128, CCHUNK) tiles, multiplies by the broadcast weight on the vector
engine writing fp16 products in place, and reduces each 16-element segment
with a binary tree of fp16 tensor-adds (DVE 2x packed mode) whose last level
accumulates in fp32.
"""

import numpy as np

import concourse.bass as bass
import concourse.mybir as mybir
from concourse.bass_utils import run_bass_kernel_spmd
from concourse.tile import TileContext

B = 8192
IN_F = 32768
OUT_F = 2048
BLK = 16
N_CORES = 8
B_LOC = B // N_CORES  # 1024

CCHUNK = 8192               # feature columns per tile
SEG = CCHUNK // BLK         # outputs per tile (512)
N_CC = IN_F // CCHUNK       # 4
N_BT = B_LOC // 128         # 8

F32 = mybir.dt.float32
F32R = mybir.dt.float32r
F16 = mybir.dt.float16

_NC_CACHE = {}


def _build(legalize=True, **bass_kwargs):
    key = ("nc", legalize, tuple(sorted(bass_kwargs.items())))
    if key in _NC_CACHE:
        return _NC_CACHE[key]
    nc = bass.Bass(**bass_kwargs)
    x = nc.declare_dram_parameter("x", [B_LOC, IN_F], F32, isOutput=False)
    w = nc.declare_dram_parameter("weight", [OUT_F, BLK], F32R, isOutput=False)
    onesr = nc.declare_dram_parameter("onesr", [1, 128], F32R, isOutput=False)
    y = nc.declare_dram_parameter("y", [B_LOC, OUT_F], F32, isOutput=True)

    wf = w[:].rearrange("o k -> (o k)")  # (32768,) flat, f = o*16 + k

    with TileContext(nc) as tc:
        with (
            tc.tile_pool(name="wpool", bufs=2) as wpool,
            tc.tile_pool(name="wrowp", bufs=1) as wrowp,
            tc.tile_pool(name="xpool", bufs=3) as xpool,
            tc.tile_pool(name="ypool", bufs=4) as ypool,
            tc.tile_pool(name="probe", bufs=2) as probepool,
            tc.tile_pool(name="const", bufs=1) as constp,
            tc.tile_pool(name="psb", bufs=2, space="PSUM") as psb,
        ):
            ones = constp.tile([1, 128], F32R)
            nc.sync.dma_start(out=ones[:], in_=onesr[:])
            for cc in range(N_CC):
                # Broadcast the weight chunk across all 128 partitions with
                # the PE: wtile[p, f] = wrow[0, f] via a K=1 ones-column
                # fp32r matmul (saves 16 MiB/core of HBM re-reads).
                wtile = wpool.tile([128, CCHUNK], F32)
                for h in range(2):
                    wrow = wrowp.tile([1, CCHUNK // 2], F32R)
                    off = cc * CCHUNK + h * (CCHUNK // 2)
                    nc.sync.dma_start(out=wrow[:], in_=wf[off : off + CCHUNK // 2])
                    for s in range(CCHUNK // 2 // 512):
                        wps = psb.tile([128, 512], F32)
                        nc.tensor.matmul(
                            out=wps[:, :],
                            lhsT=ones[:, 0:128],
                            rhs=wrow[:, s * 512 : (s + 1) * 512],
                            skip_group_check=True,
                        )
                        col = h * (CCHUNK // 2) + s * 512
                        nc.scalar.copy(out=wtile[:, col : col + 512], in_=wps[:])
                # Observer: the multiplies below then carry only their x-DMA
                # wait (walrus allows one sync wait per compute instruction).
                probe = probepool.tile([1, 1], F32)
                nc.vector.tensor_copy(out=probe[:], in_=wtile[0:1, 0:1])
                for bt in range(N_BT):
                    xtile = xpool.tile([128, CCHUNK], F32)
                    nc.sync.dma_start(
                        out=xtile[:],
                        in_=x[bt * 128 : (bt + 1) * 128, cc * CCHUNK : (cc + 1) * CCHUNK],
                    )
                    # Multiply, writing fp16 products into the tile's own
                    # first half (write offset 2i trails read offset 4i, so
                    # the in-place overlap is stream-safe).
                    prod = xtile[:, 0 : CCHUNK // 2].bitcast(F16)  # (128, 8192) f16
                    nc.vector.tensor_mul(out=prod, in0=xtile[:], in1=wtile[:])
                    # Segmented 16 -> 1 reduction as a binary tree. fp16
                    # levels run in the DVE 2x packed mode; the final level
                    # accumulates into fp32.
                    p3 = prod.rearrange("p (s k) -> p s k", k=16)
                    c0 = CCHUNK // 2
                    l1o = xtile[:, c0 : c0 + CCHUNK // 4].bitcast(F16)
                    l1 = l1o.rearrange("p (s k) -> p s k", k=8)
                    nc.vector.tensor_add(
                        out=l1, in0=p3[:, :, 0:8], in1=p3[:, :, 8:16]
                    )
                    c1 = c0 + CCHUNK // 4
                    l2o = xtile[:, c1 : c1 + CCHUNK // 8].bitcast(F16)
                    l2 = l2o.rearrange("p (s k) -> p s k", k=4)
                    nc.vector.tensor_add(out=l2, in0=l1[:, :, 0:4], in1=l1[:, :, 4:8])
                    c2 = c1 + CCHUNK // 8
                    l3o = xtile[:, c2 : c2 + CCHUNK // 16].bitcast(F16)
                    l3 = l3o.rearrange("p (s k) -> p s k", k=2)
                    nc.vector.tensor_add(out=l3, in0=l2[:, :, 0:2], in1=l2[:, :, 2:4])
                    ytile = ypool.tile([128, SEG], F32)
                    nc.vector.tensor_add(
                        out=ytile[:], in0=l3[:, :, 0], in1=l3[:, :, 1]
                    )
                    nc.sync.dma_start(
                        out=y[bt * 128 : (bt + 1) * 128, cc * SEG : (cc + 1) * SEG],
                        in_=ytile[:],
                    )
    if legalize:
        _legalize_waits(nc)
        _audit_waits(nc)
    _NC_CACHE[key] = nc
    return nc


_ES_COUNTER = [0]


def _legalize_waits(nc):
    """walrus (this CoreV3 pin) accepts one sync wait per instruction (two on
    EventSemaphore); Tile sometimes emits more. Two fixes, in order:
      1. drop same-engine self-waits (a serial engine already executes its
         own stream in order, so a wait on its own proc lane is redundant);
      2. hoist still-excess waits onto EventSemaphore instructions inserted
         right before the offender on the same engine queue.
    """
    for b in nc.m.functions[0].blocks:
        il = b.instructions
        idx = 0
        while idx < len(il):
            i = il[idx]
            si = i.sync_info
            cap = 2 if i.opcode == "EventSemaphore" else 1
            if si is None or len(si.on_wait) <= cap:
                idx += 1
                continue
            eng = str(i.engine).split(".")[-1]
            keeps = []
            for w in si.on_wait:
                rest = None
                if w.ant_name.startswith(f"{eng}_sequencer_"):
                    rest = w.ant_name[len(eng) + 11 :]
                elif w.ant_name.startswith(f"{eng}_"):
                    rest = w.ant_name[len(eng) + 1 :]
                if rest is not None and rest.isdigit():
                    continue  # self-wait: implied by program order
                keeps.append(w)
            hoist, tail = keeps[:-cap], keeps[-cap:]
            while hoist:
                chunk, hoist = hoist[:2], hoist[2:]
                _ES_COUNTER[0] += 1
                es = mybir.InstEventSemaphore(
                    name=f"legalize-es-{_ES_COUNTER[0]}", ins=[], outs=[]
                )
                es.engine = i.engine
                es.sync_info = mybir.SyncInfo(on_wait=chunk, on_update=[])
                il.insert(idx, es)
                idx += 1
            i.sync_info = mybir.SyncInfo(on_wait=tail, on_update=list(si.on_update))
            idx += 1


def _audit_waits(nc):
    """walrus (CoreV3) accepts at most one sync wait per instruction
    (two on EventSemaphore). Fail at build time instead of compile time."""
    bad = []
    for b in nc.m.functions[0].blocks:
        for i in b.instructions:
            si = i.sync_info
            if si is None:
                continue
            cap = 2 if i.opcode == "EventSemaphore" else 1
            if len(si.on_wait) > cap:
                bad.append((i.name, i.opcode, len(si.on_wait)))
    if bad:
        raise AssertionError(f"instructions with too many waits: {bad[:10]}")


def _in_maps(x, weight):
    x = np.ascontiguousarray(np.asarray(x, dtype=np.float32))
    weight = np.ascontiguousarray(np.asarray(weight, dtype=np.float32))
    ones = np.ones((1, 128), dtype=np.float32)
    return [
        {"x": x[i * B_LOC : (i + 1) * B_LOC], "weight": weight, "onesr": ones}
        for i in range(N_CORES)
    ]


def run(x, weight, **spmd_kwargs):
    nc = _build()
    res = run_bass_kernel_spmd(
        nc, _in_maps(x, weight), core_ids=list(range(N_CORES)), **spmd_kwargs
    )
    out = np.concatenate([r["y"] for r in res.results], axis=0)
    return out, res


def kernel(x, weight):
    out, _ = run(x, weight)
    return out


# revision 14
# speedup vs baseline: 1.1251x; 1.1251x over previous
"""Block-diagonal linear (segment_reduce) Trainium2 kernel.

y[b, o] = sum_k x[b, o*16 + k] * weight[o, k]
x: (8192, 32768) f32, weight: (2048, 16) f32 -> y: (8192, 2048) f32

Sharding: data-parallel over batch across 8 NeuronCores (1024 rows each);
weight replicated (broadcast across partitions on-chip by the otherwise-idle
TensorE instead of re-reading it 128x from HBM). Per core the kernel streams
x in (128, CCHUNK) tiles, multiplies by the broadcast weight on the vector
engine writing fp16 products in place, and reduces each 16-element segment
with a binary tree of fp16 tensor-adds (DVE 2x packed mode) whose last level
accumulates in fp32.
"""

import numpy as np

import concourse.bass as bass
import concourse.mybir as mybir
from concourse.bass_utils import run_bass_kernel_spmd
from concourse.tile import TileContext

B = 8192
IN_F = 32768
OUT_F = 2048
BLK = 16
N_CORES = 8
B_LOC = B // N_CORES  # 1024

CCHUNK = 8192               # feature columns per tile
SEG = CCHUNK // BLK         # outputs per tile (512)
N_CC = IN_F // CCHUNK       # 4
N_BT = B_LOC // 128         # 8

F32 = mybir.dt.float32
F32R = mybir.dt.float32r
F16 = mybir.dt.float16

_NC_CACHE = {}


def _build(legalize=True, **bass_kwargs):
    key = ("nc", legalize, tuple(sorted(bass_kwargs.items())))
    if key in _NC_CACHE:
        return _NC_CACHE[key]
    nc = bass.Bass(**bass_kwargs)
    x = nc.declare_dram_parameter("x", [B_LOC, IN_F], F32, isOutput=False)
    w = nc.declare_dram_parameter("weight", [OUT_F, BLK], F32R, isOutput=False)
    onesr = nc.declare_dram_parameter("onesr", [1, 128], F32R, isOutput=False)
    y = nc.declare_dram_parameter("y", [B_LOC, OUT_F], F32, isOutput=True)

    wf = w[:].rearrange("o k -> (o k)")  # (32768,) flat, f = o*16 + k

    with TileContext(nc) as tc:
        with (
            tc.tile_pool(name="wpool", bufs=2) as wpool,
            tc.tile_pool(name="wrowp", bufs=1) as wrowp,
            tc.tile_pool(name="xpool", bufs=4) as xpool,
            tc.tile_pool(name="ypool", bufs=4) as ypool,
            tc.tile_pool(name="probe", bufs=2) as probepool,
            tc.tile_pool(name="const", bufs=1) as constp,
            tc.tile_pool(name="psb", bufs=2, space="PSUM") as psb,
        ):
            ones = constp.tile([1, 128], F32R)
            nc.sync.dma_start(out=ones[:], in_=onesr[:])
            for cc in range(N_CC):
                # Broadcast the weight chunk across all 128 partitions with
                # the PE: wtile[p, f] = wrow[0, f] via a K=1 ones-column
                # fp32r matmul (saves 16 MiB/core of HBM re-reads). The
                # psum->sbuf copy casts to fp16 to match the x tiles.
                wtile = wpool.tile([128, CCHUNK], F16)
                for h in range(2):
                    wrow = wrowp.tile([1, CCHUNK // 2], F32R)
                    off = cc * CCHUNK + h * (CCHUNK // 2)
                    nc.sync.dma_start(out=wrow[:], in_=wf[off : off + CCHUNK // 2])
                    for s in range(CCHUNK // 2 // 512):
                        wps = psb.tile([128, 512], F32)
                        nc.tensor.matmul(
                            out=wps[:, :],
                            lhsT=ones[:, 0:128],
                            rhs=wrow[:, s * 512 : (s + 1) * 512],
                            skip_group_check=True,
                        )
                        col = h * (CCHUNK // 2) + s * 512
                        nc.scalar.copy(out=wtile[:, col : col + 512], in_=wps[:])
                # Observer: the multiplies below then carry only their x-DMA
                # wait (walrus allows one sync wait per compute instruction).
                probe = probepool.tile([1, 1], F32)
                nc.vector.tensor_copy(out=probe[:], in_=wtile[0:1, 0:1])
                for bt in range(N_BT):
                    # SWDGE DMA casts x to fp16 on the way in, so the
                    # multiply runs in the DVE 2x packed mode.
                    xtile = xpool.tile([128, CCHUNK], F16)
                    nc.gpsimd.dma_start(
                        out=xtile[:],
                        in_=x[bt * 128 : (bt + 1) * 128, cc * CCHUNK : (cc + 1) * CCHUNK],
                    )
                    nc.vector.tensor_mul(out=xtile[:], in0=xtile[:], in1=wtile[:])
                    # Segmented 16 -> 1 reduction as a binary tree that
                    # telescopes in place (each level's writes trail its
                    # reads); the final level accumulates into fp32.
                    p3 = xtile[:].rearrange("p (s k) -> p s k", k=16)
                    l1 = xtile[:, 0 : CCHUNK // 2].rearrange("p (s k) -> p s k", k=8)
                    nc.vector.tensor_add(
                        out=l1, in0=p3[:, :, 0:8], in1=p3[:, :, 8:16]
                    )
                    l2 = xtile[:, 0 : CCHUNK // 4].rearrange("p (s k) -> p s k", k=4)
                    nc.vector.tensor_add(out=l2, in0=l1[:, :, 0:4], in1=l1[:, :, 4:8])
                    l3 = xtile[:, 0 : CCHUNK // 8].rearrange("p (s k) -> p s k", k=2)
                    nc.vector.tensor_add(out=l3, in0=l2[:, :, 0:2], in1=l2[:, :, 2:4])
                    ytile = ypool.tile([128, SEG], F32)
                    nc.vector.tensor_add(
                        out=ytile[:], in0=l3[:, :, 0], in1=l3[:, :, 1]
                    )
                    nc.sync.dma_start(
                        out=y[bt * 128 : (bt + 1) * 128, cc * SEG : (cc + 1) * SEG],
                        in_=ytile[:],
                    )
    if legalize:
        _legalize_waits(nc)
        _audit_waits(nc)
    _NC_CACHE[key] = nc
    return nc


_ES_COUNTER = [0]


def _legalize_waits(nc):
    """walrus (this CoreV3 pin) accepts one sync wait per instruction (two on
    EventSemaphore); Tile sometimes emits more. Two fixes, in order:
      1. drop same-engine self-waits (a serial engine already executes its
         own stream in order, so a wait on its own proc lane is redundant);
      2. hoist still-excess waits onto EventSemaphore instructions inserted
         right before the offender on the same engine queue.
    """
    for b in nc.m.functions[0].blocks:
        il = b.instructions
        idx = 0
        while idx < len(il):
            i = il[idx]
            si = i.sync_info
            cap = 2 if i.opcode == "EventSemaphore" else 1
            if si is None or len(si.on_wait) <= cap:
                idx += 1
                continue
            eng = str(i.engine).split(".")[-1]
            keeps = []
            for w in si.on_wait:
                rest = None
                if w.ant_name.startswith(f"{eng}_sequencer_"):
                    rest = w.ant_name[len(eng) + 11 :]
                elif w.ant_name.startswith(f"{eng}_"):
                    rest = w.ant_name[len(eng) + 1 :]
                if rest is not None and rest.isdigit():
                    continue  # self-wait: implied by program order
                keeps.append(w)
            hoist, tail = keeps[:-cap], keeps[-cap:]
            while hoist:
                chunk, hoist = hoist[:2], hoist[2:]
                _ES_COUNTER[0] += 1
                es = mybir.InstEventSemaphore(
                    name=f"legalize-es-{_ES_COUNTER[0]}", ins=[], outs=[]
                )
                es.engine = i.engine
                es.sync_info = mybir.SyncInfo(on_wait=chunk, on_update=[])
                il.insert(idx, es)
                idx += 1
            i.sync_info = mybir.SyncInfo(on_wait=tail, on_update=list(si.on_update))
            idx += 1


def _audit_waits(nc):
    """walrus (CoreV3) accepts at most one sync wait per instruction
    (two on EventSemaphore). Fail at build time instead of compile time."""
    bad = []
    for b in nc.m.functions[0].blocks:
        for i in b.instructions:
            si = i.sync_info
            if si is None:
                continue
            cap = 2 if i.opcode == "EventSemaphore" else 1
            if len(si.on_wait) > cap:
                bad.append((i.name, i.opcode, len(si.on_wait)))
    if bad:
        raise AssertionError(f"instructions with too many waits: {bad[:10]}")


def _in_maps(x, weight):
    x = np.ascontiguousarray(np.asarray(x, dtype=np.float32))
    weight = np.ascontiguousarray(np.asarray(weight, dtype=np.float32))
    ones = np.ones((1, 128), dtype=np.float32)
    return [
        {"x": x[i * B_LOC : (i + 1) * B_LOC], "weight": weight, "onesr": ones}
        for i in range(N_CORES)
    ]


def run(x, weight, **spmd_kwargs):
    nc = _build()
    res = run_bass_kernel_spmd(
        nc, _in_maps(x, weight), core_ids=list(range(N_CORES)), **spmd_kwargs
    )
    out = np.concatenate([r["y"] for r in res.results], axis=0)
    return out, res


def kernel(x, weight):
    out, _ = run(x, weight)
    return out


# revision 15
# speedup vs baseline: 1.2290x; 1.0923x over previous
"""Block-diagonal linear (segment_reduce) Trainium2 kernel.

y[b, o] = sum_k x[b, o*16 + k] * weight[o, k]
x: (8192, 32768) f32, weight: (2048, 16) f32 -> y: (8192, 2048) f32

Sharding: data-parallel over batch across 8 NeuronCores (1024 rows each);
weight replicated (broadcast across partitions on-chip by the otherwise-idle
TensorE instead of re-reading it 128x from HBM). Per core the kernel streams
x in (128, CCHUNK) tiles, multiplies by the broadcast weight on the vector
engine writing fp16 products in place, and reduces each 16-element segment
with a binary tree of fp16 tensor-adds (DVE 2x packed mode) whose last level
accumulates in fp32.
"""

import numpy as np

import concourse.bass as bass
import concourse.mybir as mybir
from concourse.bass_utils import run_bass_kernel_spmd
from concourse.tile import TileContext

B = 8192
IN_F = 32768
OUT_F = 2048
BLK = 16
N_CORES = 8
B_LOC = B // N_CORES  # 1024

CCHUNK = 16384              # feature columns per tile
SEG = CCHUNK // BLK         # outputs per tile (512)
N_CC = IN_F // CCHUNK       # 4
N_BT = B_LOC // 128         # 8

F32 = mybir.dt.float32
F32R = mybir.dt.float32r
F16 = mybir.dt.float16

_NC_CACHE = {}


def _build(legalize=True, **bass_kwargs):
    key = ("nc", legalize, tuple(sorted(bass_kwargs.items())))
    if key in _NC_CACHE:
        return _NC_CACHE[key]
    nc = bass.Bass(**bass_kwargs)
    x = nc.declare_dram_parameter("x", [B_LOC, IN_F], F32, isOutput=False)
    w = nc.declare_dram_parameter("weight", [OUT_F, BLK], F32R, isOutput=False)
    onesr = nc.declare_dram_parameter("onesr", [1, 128], F32R, isOutput=False)
    y = nc.declare_dram_parameter("y", [B_LOC, OUT_F], F32, isOutput=True)

    wf = w[:].rearrange("o k -> (o k)")  # (32768,) flat, f = o*16 + k

    with TileContext(nc) as tc:
        with (
            tc.tile_pool(name="wpool", bufs=2) as wpool,
            tc.tile_pool(name="wrowp", bufs=1) as wrowp,
            tc.tile_pool(name="xpool", bufs=3) as xpool,
            tc.tile_pool(name="ypool", bufs=4) as ypool,
            tc.tile_pool(name="probe", bufs=2) as probepool,
            tc.tile_pool(name="const", bufs=1) as constp,
            tc.tile_pool(name="psb", bufs=2, space="PSUM") as psb,
        ):
            ones = constp.tile([1, 128], F32R)
            nc.sync.dma_start(out=ones[:], in_=onesr[:])
            for cc in range(N_CC):
                # Broadcast the weight chunk across all 128 partitions with
                # the PE: wtile[p, f] = wrow[0, f] via a K=1 ones-column
                # fp32r matmul (saves 16 MiB/core of HBM re-reads). The
                # psum->sbuf copy casts to fp16 to match the x tiles.
                wtile = wpool.tile([128, CCHUNK], F16)
                for h in range(4):
                    wrow = wrowp.tile([1, CCHUNK // 4], F32R)
                    off = cc * CCHUNK + h * (CCHUNK // 4)
                    nc.sync.dma_start(out=wrow[:], in_=wf[off : off + CCHUNK // 4])
                    for s in range(CCHUNK // 4 // 512):
                        wps = psb.tile([128, 512], F32)
                        nc.tensor.matmul(
                            out=wps[:, :],
                            lhsT=ones[:, 0:128],
                            rhs=wrow[:, s * 512 : (s + 1) * 512],
                            skip_group_check=True,
                        )
                        col = h * (CCHUNK // 4) + s * 512
                        nc.scalar.copy(out=wtile[:, col : col + 512], in_=wps[:])
                # Observer: the multiplies below then carry only their x-DMA
                # wait (walrus allows one sync wait per compute instruction).
                probe = probepool.tile([1, 1], F32)
                nc.vector.tensor_copy(out=probe[:], in_=wtile[0:1, 0:1])
                for bt in range(N_BT):
                    # SWDGE DMA casts x to fp16 on the way in, so the
                    # multiply runs in the DVE 2x packed mode.
                    xtile = xpool.tile([128, CCHUNK], F16)
                    nc.gpsimd.dma_start(
                        out=xtile[:],
                        in_=x[bt * 128 : (bt + 1) * 128, cc * CCHUNK : (cc + 1) * CCHUNK],
                    )
                    nc.vector.tensor_mul(out=xtile[:], in0=xtile[:], in1=wtile[:])
                    # Segmented 16 -> 1 reduction as a binary tree that
                    # telescopes in place (each level's writes trail its
                    # reads); the final level accumulates into fp32.
                    p3 = xtile[:].rearrange("p (s k) -> p s k", k=16)
                    l1 = xtile[:, 0 : CCHUNK // 2].rearrange("p (s k) -> p s k", k=8)
                    nc.vector.tensor_add(
                        out=l1, in0=p3[:, :, 0:8], in1=p3[:, :, 8:16]
                    )
                    l2 = xtile[:, 0 : CCHUNK // 4].rearrange("p (s k) -> p s k", k=4)
                    nc.vector.tensor_add(out=l2, in0=l1[:, :, 0:4], in1=l1[:, :, 4:8])
                    l3 = xtile[:, 0 : CCHUNK // 8].rearrange("p (s k) -> p s k", k=2)
                    nc.vector.tensor_add(out=l3, in0=l2[:, :, 0:2], in1=l2[:, :, 2:4])
                    ytile = ypool.tile([128, SEG], F32)
                    nc.vector.tensor_add(
                        out=ytile[:], in0=l3[:, :, 0], in1=l3[:, :, 1]
                    )
                    nc.sync.dma_start(
                        out=y[bt * 128 : (bt + 1) * 128, cc * SEG : (cc + 1) * SEG],
                        in_=ytile[:],
                    )
    if legalize:
        _legalize_waits(nc)
        _audit_waits(nc)
    _NC_CACHE[key] = nc
    return nc


_ES_COUNTER = [0]


def _legalize_waits(nc):
    """walrus (this CoreV3 pin) accepts one sync wait per instruction (two on
    EventSemaphore); Tile sometimes emits more. Two fixes, in order:
      1. drop same-engine self-waits (a serial engine already executes its
         own stream in order, so a wait on its own proc lane is redundant);
      2. hoist still-excess waits onto EventSemaphore instructions inserted
         right before the offender on the same engine queue.
    """
    for b in nc.m.functions[0].blocks:
        il = b.instructions
        idx = 0
        while idx < len(il):
            i = il[idx]
            si = i.sync_info
            cap = 2 if i.opcode == "EventSemaphore" else 1
            if si is None or len(si.on_wait) <= cap:
                idx += 1
                continue
            eng = str(i.engine).split(".")[-1]
            keeps = []
            for w in si.on_wait:
                rest = None
                if w.ant_name.startswith(f"{eng}_sequencer_"):
                    rest = w.ant_name[len(eng) + 11 :]
                elif w.ant_name.startswith(f"{eng}_"):
                    rest = w.ant_name[len(eng) + 1 :]
                if rest is not None and rest.isdigit():
                    continue  # self-wait: implied by program order
                keeps.append(w)
            hoist, tail = keeps[:-cap], keeps[-cap:]
            while hoist:
                chunk, hoist = hoist[:2], hoist[2:]
                _ES_COUNTER[0] += 1
                es = mybir.InstEventSemaphore(
                    name=f"legalize-es-{_ES_COUNTER[0]}", ins=[], outs=[]
                )
                es.engine = i.engine
                es.sync_info = mybir.SyncInfo(on_wait=chunk, on_update=[])
                il.insert(idx, es)
                idx += 1
            i.sync_info = mybir.SyncInfo(on_wait=tail, on_update=list(si.on_update))
            idx += 1


def _audit_waits(nc):
    """walrus (CoreV3) accepts at most one sync wait per instruction
    (two on EventSemaphore). Fail at build time instead of compile time."""
    bad = []
    for b in nc.m.functions[0].blocks:
        for i in b.instructions:
            si = i.sync_info
            if si is None:
                continue
            cap = 2 if i.opcode == "EventSemaphore" else 1
            if len(si.on_wait) > cap:
                bad.append((i.name, i.opcode, len(si.on_wait)))
    if bad:
        raise AssertionError(f"instructions with too many waits: {bad[:10]}")


def _in_maps(x, weight):
    x = np.ascontiguousarray(np.asarray(x, dtype=np.float32))
    weight = np.ascontiguousarray(np.asarray(weight, dtype=np.float32))
    ones = np.ones((1, 128), dtype=np.float32)
    return [
        {"x": x[i * B_LOC : (i + 1) * B_LOC], "weight": weight, "onesr": ones}
        for i in range(N_CORES)
    ]


def run(x, weight, **spmd_kwargs):
    nc = _build()
    res = run_bass_kernel_spmd(
        nc, _in_maps(x, weight), core_ids=list(range(N_CORES)), **spmd_kwargs
    )
    out = np.concatenate([r["y"] for r in res.results], axis=0)
    return out, res


def kernel(x, weight):
    out, _ = run(x, weight)
    return out
